# revision 1
# baseline (speedup 1.0000x reference)
"""Trainium2 Bass kernel for nn_LiveNet_20504173871714 (dense MLP).

    out = relu(relu(x @ W1.T + b1) @ W2.T + b2)
    x: [4096, 2048] f32, W1: [8192, 2048], W2: [2048, 8192]

Strategy: data-parallel over batch across 8 NeuronCores (512 rows each);
no collectives. On each core the whole pipeline runs in a "transposed"
flow so that every DMA is contiguous and no on-device transpose is needed:

    GEMM1: hiddenT[m, b] = sum_i W1T[i, m] * xT[i, b]   (W1 tiles stationary)
    GEMM2: outT[o, b]    = sum_j W2T[j, o] * hiddenT[j, b]

The host pre-transposes x/W1/W2 (cheap numpy work) and transposes the
per-core out shards back at the end. GEMM1 runs in float32r (1 cyc/row,
keeps x precision — x rounding is the dominant, non-averaging error
term); GEMM2 runs in fp8e4m3 DoubleRow (0.5 cyc/row) with hidden as the
stationary operand so each non-FWL weight load is shared by 4 matmuls;
accumulation is fp32 in PSUM. ReLU+bias eviction runs on ScalarE.
"""

import numpy as np
import ml_dtypes

N_IN, N_MID, N_OUT, BATCH = 2048, 8192, 2048, 4096
N_CORES = 8
B = BATCH // N_CORES  # 512 rows per core
P = 128

# Matmul operand dtypes (mybir names): "bfloat16", "float32r", or for
# GEMM2 additionally "fp8dr" (float8e4m3 with DoubleRow perf mode).
G1_DT = "float32r"
G2_DT = "fp8drf"

_CACHE = {}


def _np_dt(mybir_name):
    return {
        "bfloat16": ml_dtypes.bfloat16,
        "float32r": np.float32,
        "float32": np.float32,
        "fp8dr": ml_dtypes.float8_e4m3,
        "fp8drf": ml_dtypes.float8_e4m3,
    }[mybir_name]


def _build(g1_dt=G1_DT, g2_dt=G2_DT, enable_asserts=False, reps=1):
    """Build + compile the per-core Bass module (cached).

    reps>1 repeats the whole computation sequentially inside one NEFF —
    used only for timing (amortizes the per-call dispatch overhead)."""
    key = (g1_dt, g2_dt, enable_asserts, reps)
    if key in _CACHE:
        return _CACHE[key]

    import concourse.bass as bass
    import concourse.mybir as mybir
    import concourse.tile as tile
    from concourse import bacc
    from concourse.bass import ds, ts
    from contextlib import ExitStack

    g2_dr = g2_dt in ("fp8dr", "fp8drf")
    # "fp8drf": hidden is the stationary operand and W2 the moving one, so
    # each 256-col DoubleRow weight load is shared by 4 matmuls (the
    # dominant DR cost is the non-FWL weight load). Output comes out in
    # natural [B, N_OUT] orientation. Requires b2 == 0 (no bias port on the
    # free dim); kernel() falls back to "fp8dr" when b2 != 0.
    g2_flip = g2_dt == "fp8drf"
    d1 = getattr(mybir.dt, g1_dt)
    d2 = mybir.dt.float8e4 if g2_dr else getattr(mybir.dt, g2_dt)
    f32 = mybir.dt.float32
    relu = mybir.ActivationFunctionType.Relu

    nc = bacc.Bacc("TRN2", target_bir_lowering=False, debug=False,
                   enable_asserts=enable_asserts)

    xT = nc.dram_tensor("xT", [N_IN, B], d1, kind="ExternalInput").ap()
    w1T = nc.dram_tensor("w1T", [N_IN, N_MID], d1, kind="ExternalInput").ap()
    if g2_dr:
        # W2^T pre-interleaved on the host for DoubleRow:
        # w2T[t, p, q, o] = W2[o, t*256 + q*128 + p]
        w2T = nc.dram_tensor("w2T", [N_MID // (2 * P), P, 2, N_OUT], d2,
                             kind="ExternalInput").ap()
    else:
        w2T = nc.dram_tensor("w2T", [N_MID, N_OUT], d2,
                             kind="ExternalInput").ap()
    b1s = nc.dram_tensor("b1s", [P, N_MID // P], f32, kind="ExternalInput").ap()
    b2s = nc.dram_tensor("b2s", [P, N_OUT // P], f32, kind="ExternalInput").ap()
    if g2_flip:
        outT = nc.dram_tensor("outF", [B, N_OUT], f32,
                              kind="ExternalOutput").ap()
    else:
        outT = nc.dram_tensor("outT", [N_OUT, B], f32,
                              kind="ExternalOutput").ap()

    IT = N_IN // P    # 16 k-tiles in GEMM1
    JT = N_MID // P   # 64 k-tiles in GEMM2
    MG = 4            # m/o tiles per PSUM group

    with tile.TileContext(nc) as tc, ExitStack() as ctx:
        const = ctx.enter_context(tc.tile_pool(name="const", bufs=1))
        xpool = ctx.enter_context(tc.tile_pool(name="xpool", bufs=IT))
        hpool = ctx.enter_context(
            tc.tile_pool(name="hpool", bufs=(JT // 2 if g2_dr else JT)))
        w1pool = ctx.enter_context(tc.tile_pool(name="w1pool", bufs=12))
        w2pool = ctx.enter_context(tc.tile_pool(name="w2pool", bufs=12))
        opool = ctx.enter_context(tc.tile_pool(name="opool", bufs=4))
        psum = ctx.enter_context(tc.tile_pool(name="psum", bufs=8, space="PSUM"))

        b1_sb = const.tile([P, N_MID // P], f32, name="b1_sb")
        nc.sync.dma_start(b1_sb[:], b1s[:, :])
        b2_sb = const.tile([P, N_OUT // P], f32, name="b2_sb")
        nc.sync.dma_start(b2_sb[:], b2s[:, :])

        for rep in range(reps):
            # x^T resident in SBUF: 16 tiles [128, 512]
            xts = []
            for it in range(IT):
                t = xpool.tile([P, B], d1, tag="xT", name=f"xT_{it}")
                nc.sync.dma_start(t[:], xT[ts(it, P), :])
                xts.append(t)

            # GEMM1 + ReLU -> hiddenT in SBUF. For the DoubleRow GEMM2 the
            # hidden tiles are [128, 2, 512] fp8 "pair" tiles (m-tiles 2t,
            # 2t+1 interleaved on the q axis); otherwise plain [128, 512].
            if g2_dr:
                hts = [hpool.tile([P, 2, B], d2, tag="hid", name=f"hid_{t}")
                       for t in range(JT // 2)]
            else:
                hts = [hpool.tile([P, B], d2, tag="hid", name=f"hid_{t}")
                       for t in range(JT)]
            for mtg in range(N_MID // (MG * P)):
                psums = [psum.tile([P, B], f32, tag="ps", name=f"ps1_{mtg}_{s}")
                         for s in range(MG)]
                for it in range(IT):
                    blk = w1pool.tile([P, MG * P], d1, tag="w1",
                                      name=f"w1_{mtg}_{it}")
                    nc.sync.dma_start(blk[:],
                                      w1T[ts(it, P), ds(mtg * MG * P, MG * P)])
                    for s in range(MG):
                        nc.tensor.matmul(psums[s][:], blk[:, ts(s, P)],
                                         xts[it][:],
                                         start=(it == 0), stop=(it == IT - 1))
                for s in range(MG):
                    mt = mtg * MG + s
                    if g2_dr:
                        h_out = hts[mt // 2][:, mt % 2, :]
                    else:
                        h_out = hts[mt][:]
                    nc.scalar.activation(h_out, psums[s][:], relu,
                                         bias=b1_sb[:, mt:mt + 1])

            # GEMM2 + ReLU -> out
            if g2_flip:
                # hidden stationary / W2 moving; psum tiles are [b, o].
                # b (512) is split in two halves of 2 b-subtiles so the 8
                # live psum banks fit; W2 streams once per half (32 MiB).
                for bsh in range(2):
                    psums = [psum.tile([P, MG * P], f32, tag="ps",
                                       name=f"psf_{bsh}_{k}")
                             for k in range(8)]
                    for t in range(JT // 2):
                        w2t = w2pool.tile([P, 2, N_OUT], d2, tag="w2",
                                          name=f"w2_{bsh}_{t}")
                        nc.sync.dma_start(w2t[:], w2T[t, :, :, :])
                        for bi in range(2):
                            lhs = hts[t][:, :, ts(bsh * 2 + bi, P)]
                            for ob in range(4):
                                nc.tensor.matmul(
                                    psums[bi * 4 + ob][:], lhs,
                                    w2t[:, :, ds(ob * MG * P, MG * P)],
                                    start=(t == 0), stop=(t == JT // 2 - 1),
                                    perf_mode=mybir.MatmulPerfMode.DoubleRow)
                    for bi in range(2):
                        for ob in range(4):
                            o_sb = opool.tile([P, MG * P], f32, tag="out",
                                              name=f"out_{bsh}_{bi}_{ob}")
                            nc.scalar.activation(o_sb[:],
                                                 psums[bi * 4 + ob][:], relu)
                            nc.sync.dma_start(
                                outT[ds((bsh * 2 + bi) * P, P),
                                     ds(ob * MG * P, MG * P)],
                                o_sb[:])
                continue

            KT2 = JT // 2 if g2_dr else JT
            for otg in range(N_OUT // (MG * P)):
                psums = [psum.tile([P, B], f32, tag="ps", name=f"ps2_{otg}_{s}")
                         for s in range(MG)]
                for jt in range(KT2):
                    if g2_dr:
                        blk = w2pool.tile([P, 2, MG * P], d2, tag="w2",
                                          name=f"w2_{otg}_{jt}")
                        nc.sync.dma_start(
                            blk[:], w2T[jt, :, :, ds(otg * MG * P, MG * P)])
                        for s in range(MG):
                            nc.tensor.matmul(
                                psums[s][:], blk[:, :, ts(s, P)], hts[jt][:],
                                start=(jt == 0), stop=(jt == KT2 - 1),
                                perf_mode=mybir.MatmulPerfMode.DoubleRow)
                    else:
                        blk = w2pool.tile([P, MG * P], d2, tag="w2",
                                          name=f"w2_{otg}_{jt}")
                        nc.sync.dma_start(
                            blk[:], w2T[ts(jt, P), ds(otg * MG * P, MG * P)])
                        for s in range(MG):
                            nc.tensor.matmul(psums[s][:], blk[:, ts(s, P)],
                                             hts[jt][:],
                                             start=(jt == 0),
                                             stop=(jt == KT2 - 1))
                for s in range(MG):
                    ot = otg * MG + s
                    o_sb = opool.tile([P, B], f32, tag="out", name=f"out_{ot}")
                    nc.scalar.activation(o_sb[:], psums[s][:], relu,
                                         bias=b2_sb[:, ot:ot + 1])
                    nc.sync.dma_start(outT[ts(ot, P), :], o_sb[:])

    nc.compile()
    _CACHE[key] = nc
    return nc


def _prep_inputs(x, W1, b1, W2, b2, g1_dt=G1_DT, g2_dt=G2_DT):
    nd1 = _np_dt(g1_dt)
    nd2 = _np_dt(g2_dt)
    x = np.asarray(x, dtype=np.float32)
    W1T = np.ascontiguousarray(np.asarray(W1, dtype=np.float32).T.astype(nd1))
    W2Tf = np.asarray(W2, dtype=np.float32).T.astype(nd2)  # [N_MID, N_OUT]
    if g2_dt in ("fp8dr", "fp8drf"):
        # [t, p, q, o] with j = t*256 + q*128 + p
        W2T = np.ascontiguousarray(
            W2Tf.reshape(N_MID // (2 * P), 2, P, N_OUT).transpose(0, 2, 1, 3))
    else:
        W2T = np.ascontiguousarray(W2Tf)
    b1s = np.ascontiguousarray(
        np.asarray(b1, dtype=np.float32).reshape(N_MID // P, P).T)
    b2s = np.ascontiguousarray(
        np.asarray(b2, dtype=np.float32).reshape(N_OUT // P, P).T)
    in_maps = []
    for c in range(N_CORES):
        xTc = np.ascontiguousarray(x[c * B:(c + 1) * B].T.astype(nd1))
        in_maps.append({"xT": xTc, "w1T": W1T, "w2T": W2T,
                        "b1s": b1s, "b2s": b2s})
    return in_maps


def _run(x, W1, b1, W2, b2, trace=False, g1_dt=G1_DT, g2_dt=G2_DT):
    from concourse.bass_utils import run_bass_kernel_spmd
    if g2_dt == "fp8drf" and np.any(np.asarray(b2)):
        g2_dt = "fp8dr"  # flipped path has no b2 port; b2==0 in practice
    nc = _build(g1_dt, g2_dt)
    in_maps = _prep_inputs(x, W1, b1, W2, b2, g1_dt, g2_dt)
    res = run_bass_kernel_spmd(nc, in_maps, core_ids=list(range(N_CORES)),
                               trace=trace)
    if g2_dt == "fp8drf":
        out = np.concatenate(
            [res.results[c]["outF"] for c in range(N_CORES)], axis=0)
    else:
        out = np.concatenate(
            [res.results[c]["outT"].T for c in range(N_CORES)], axis=0)
    return np.ascontiguousarray(out, dtype=np.float32), res


def kernel(x, W1, b1, W2, b2):
    out, _ = _run(x, W1, b1, W2, b2, trace=False)
    return out

